# revision 41
# baseline (speedup 1.0000x reference)
"""Distributed Trainium2 kernel for the GNN message-passing model (v2).

Self-contained: host-side structural prep (sharding, edge sort, index
remap, fp8 quantization) + Bass/Tile SPMD kernel across 8 NeuronCores.

Math (see reference):
  logits = MLP(x1); m = 0.15 + 0.55*onehot(argmax(logits))
  r1 = (m@W1s.sum(-1))*x2            (bp1==0)
  g1 = relu(Dh A Dh (r1@gcn1_w)); g1 = (m@W12)*g1 + 2e-4*(r1@W13)
  r2 = (m@W2.sum(-1))*g1             (bp2==0)
  g2 = relu(Dh A Dh (r2@gcn2_w))
  out = log_softmax(g2@fc_w + fc_b)
where Dh = diag(deg^-0.5), deg = in-degree over dst.

v2 design vs baseline:
  * fp8(e4m3) on the heavy paths with DoubleRow perf mode (2 k-tiles
    per matmul, 0.5 cyc/row): MLP, gcn convs, and the edge scatter
    matmuls (pairs of 128-edge blocks -> one PSUM update).
  * One-hot scatter matrices, gathered feature tables, and AllGather
    payloads in fp8 (halves DMA/comm bytes); host-managed value scales
    keep everything in the e4m3 sweet spot.
  * L1 aggregation in 3 rounds by src chunk ({0,1},{2,3,4},{5,6}),
    partial sums carried in SBUF (no DRAM spill); L2 in 2 rounds.
  * L2 aggregation transposed ([G2, dstblock] PSUM) so the final FC
    needs no per-block transpose.
  * Collective-completion hazard workaround: the Collectives semaphore
    fires when a collective STARTS, so consumers of an AllGather's
    output can read stale DRAM. After each real AllGather we issue a
    tiny sentinel AllGather into spare tail rows of the same table;
    the CC engine is serial, so a consumer gated on the sentinel's
    write can only run after the real collective's data landed.
  * Issue-order interleaving: L1 round-0 scatter is issued between
    front chunks 5..6, and the L1 finalize round is interleaved into
    the DMA-bound round-1 scatter phase.
"""

import numpy as np
import ml_dtypes

P = 128
TAU_HI = 0.7
TAU_LO = 0.15  # (1-0.7)/2
F8 = ml_dtypes.float8_e4m3
BF = ml_dtypes.bfloat16

# fp8 value scales
SW = 16.0          # weight prescale (w1,w2,w3,g1w,g2w)
SH = 4.0           # MLP hidden activation scale
SR1 = 4.0          # r1 fp8 scale (folded into W1s prescale)
ST1 = 16.0         # h1' table scale
SR2 = 64.0         # r2 fp8 scale (folded into dinv_c)
SZ = 65536.0       # w13 extra prescale (on top of the 2e-4)


class _Cfg:
    def __init__(self, N, E, F1=768, H=512, G1=256, G2=32, FOUT=40, C=7):
        self.NC = 8
        self.N = N
        self.E = E
        self.NLOC_RAW = N // self.NC
        self.NB = -(-self.NLOC_RAW // P)          # node blocks per core
        self.NLOC = self.NB * P
        assert self.NB % C == 0, (self.NB, C)
        self.C = C                                 # front chunks
        self.BPC = self.NB // C                    # blocks per chunk
        self.CH = self.BPC * P                     # chunk nodes
        self.CHR = self.NC * self.CH               # rows per chunk gathered
        self.F1, self.H, self.G1, self.G2, self.FOUT = F1, H, G1, G2, FOUT
        self.KF1 = F1 // P                         # 6 k-tiles
        self.KH = H // P                           # 4
        self.KG1 = G1 // P                         # 2
        self.NFREE = min(448, self.CH)             # front free-dim unit
        assert self.CH % self.NFREE == 0
        self.FU = self.CH // self.NFREE            # free units per chunk
        self.L1R = [(0, 2), (2, 5), (5, 7)]        # L1 rounds: chunk ranges
        self.SPL2 = 5                              # L2 round A: blocks < SPL2*BPC
        self.SBP1 = 16                             # pairs per s-block, L1
        self.SBP2 = 16                             # pairs per s-block, L2


CFG_FULL = dict(N=50000, E=800000)


def _f8(x, s=1.0):
    return (np.asarray(x, np.float32) * s).astype(F8)


def _bf(x, s=1.0):
    return (np.asarray(x, np.float32) * s).astype(BF)


def build_pair_layout(cnt, NB, SBP):
    """cnt: [NC, NB] per-core edge counts for this round. Returns the
    SPMD-shared schedule (same for every core)."""
    npair = np.maximum(1, -(-cnt.max(axis=0) // 256)).astype(np.int64)
    total = int(npair.sum())
    padded = -(-total // SBP) * SBP
    b_of = np.concatenate([np.repeat(np.arange(NB), npair),
                           np.full(padded - total, NB - 1)]).astype(np.int64)
    first = np.zeros(padded, bool)
    last = np.zeros(padded, bool)
    first[0] = True
    first[1:] = b_of[1:] != b_of[:-1]
    last[:-1] = b_of[1:] != b_of[:-1]
    last[-1] = True
    pair_base = np.concatenate([[0], np.cumsum(npair)])
    return dict(npair=npair, nsb=padded // SBP, b_of=b_of, first=first,
                last=last, pair_base=pair_base)


def pack_round(layout, eb_c, SBP, NB):
    """Per-core pack: idx [nsb*P, 2*SBP] int32 and Sm [nsb*P, 2*SBP*P] fp8."""
    nsb = layout["nsb"]
    idx = np.zeros((nsb * P, 2 * SBP), np.int32)
    Sm = np.zeros((nsb * P, 2 * SBP * P), np.float32)
    pair_base = layout["pair_base"]
    for b in range(NB):
        rows, drel = eb_c[b]
        n = len(rows)
        if n == 0:
            continue
        i = np.arange(n)
        gp = pair_base[b] + i // 256
        sb = gp // SBP
        jp = gp % SBP
        col = 2 * jp + (i // 128) % 2
        r = sb * P + (i % 128)
        idx[r, col] = rows
        Sm[r, col * P + drel] = 1.0
    return idx, Sm.astype(F8)


def host_prep(inputs, cfg):
    x1 = np.asarray(inputs["x1"], np.float32)
    x2 = np.asarray(inputs["x2"], np.float32)
    ei = np.asarray(inputs["edge_index"])
    src = ei[0].astype(np.int64)
    dst = ei[1].astype(np.int64)
    N, E, NC, NB, CH, CHR = cfg.N, cfg.E, cfg.NC, cfg.NB, cfg.CH, cfg.CHR
    assert x1.shape[0] == N and src.shape[0] == E

    deg = np.bincount(dst, minlength=N).astype(np.float64)
    dinv = np.where(deg > 0, deg ** -0.5, 0.0).astype(np.float32)

    # ---- per-core edge partition by dst owner ----
    owner = dst // cfg.NLOC_RAW
    dloc = dst - owner * cfg.NLOC_RAW
    dblk = dloc // P
    drel_all = (dloc - dblk * P).astype(np.int64)

    # src decomposition
    c_s = src // cfg.NLOC_RAW
    s_s = src - c_s * cfg.NLOC_RAW
    k_s = s_s // CH  # front chunk of src

    # L1 table rows per round (chunk ranges)
    SPL2B = cfg.SPL2 * cfg.BPC * P  # local-slot split for L2 (4480)
    NL2B = cfg.NLOC - SPL2B         # 1792

    rows_l1 = []
    masks_l1 = []
    for (k0, k1) in cfg.L1R:
        m = (k_s >= k0) & (k_s < k1)
        rows_l1.append((c_s * (k1 - k0) * CH + (s_s - k0 * CH))
                       .astype(np.int32))
        masks_l1.append(m)
    # L2 rows per round (src local slot ranges)
    m2a = s_s < SPL2B
    rows_l2 = [(c_s * SPL2B + s_s).astype(np.int32),
               (c_s * NL2B + (s_s - SPL2B)).astype(np.int32)]
    masks_l2 = [m2a, ~m2a]

    def split_per_core(rows, mask):
        """-> cnt [NC, NB], percore[c][b] = (rows, drel)"""
        cnt = np.zeros((NC, NB), np.int64)
        percore = []
        for c in range(NC):
            sel = np.where((owner == c) & mask)[0]
            order = np.argsort(dblk[sel], kind="stable")
            sel = sel[order]
            b_of = dblk[sel]
            bounds = np.searchsorted(b_of, np.arange(NB + 1))
            pc = []
            for b in range(NB):
                ii = sel[bounds[b]:bounds[b + 1]]
                pc.append((rows[ii], drel_all[ii]))
                cnt[c, b] = len(ii)
            percore.append(pc)
        return cnt, percore

    lay1, pc1 = [], []
    for r in range(3):
        cnt, percore = split_per_core(rows_l1[r], masks_l1[r])
        lay1.append(build_pair_layout(cnt, NB, cfg.SBP1))
        pc1.append(percore)
    lay2, pc2 = [], []
    for r in range(2):
        cnt, percore = split_per_core(rows_l2[r], masks_l2[r])
        lay2.append(build_pair_layout(cnt, NB, cfg.SBP2))
        pc2.append(percore)

    sched = dict(
        lay1=[dict(nsb=l["nsb"], b_of=l["b_of"], first=l["first"],
                   last=l["last"]) for l in lay1],
        lay2=[dict(nsb=l["nsb"], b_of=l["b_of"], first=l["first"],
                   last=l["last"]) for l in lay2],
    )

    # ---- weights ----
    w1 = np.asarray(inputs["mlp_w1"], np.float32)
    w2 = np.asarray(inputs["mlp_w2"], np.float32)
    w3 = np.asarray(inputs["mlp_w3"], np.float32)
    b1 = np.asarray(inputs["mlp_b1"], np.float32)
    b2 = np.asarray(inputs["mlp_b2"], np.float32)
    b3 = np.asarray(inputs["mlp_b3"], np.float32)
    W1s = np.asarray(inputs["W1"], np.float32).sum(-1)
    W12 = np.asarray(inputs["W12"], np.float32)
    W13 = np.asarray(inputs["W13"], np.float32) * 2e-4
    bp1 = np.asarray(inputs["bp1"], np.float32)
    W2s = np.asarray(inputs["W2"], np.float32).sum(-1)
    bp2 = np.asarray(inputs["bp2"], np.float32)
    g1w = np.asarray(inputs["gcn1_w"], np.float32)
    g1b = np.asarray(inputs["gcn1_b"], np.float32)
    g2w = np.asarray(inputs["gcn2_w"], np.float32)
    g2b = np.asarray(inputs["gcn2_b"], np.float32)
    fcw = np.asarray(inputs["fc_w"], np.float32)
    fcb = np.asarray(inputs["fc_b"], np.float32)

    # these are structurally zero for this model; the kernel relies on it
    assert not np.any(b1 != 0) and not np.any(b2 != 0)
    assert not np.any(g1b != 0) and not np.any(g2b != 0)
    assert not np.any(fcb != 0)
    assert not np.any(bp1 != 0) and not np.any(bp2 != 0)
    sched["b3_nz"] = bool(np.any(b3 != 0))
    sched["w12_ones"] = bool(np.allclose(W12, 1.0))

    KF1, KH, G1, G2, FOUT = cfg.KF1, cfg.KH, cfg.G1, cfg.G2, cfg.FOUT

    # w1: [P, (t, m, h, P)] fp8*SW ; t over KF1/2 pairs, h in {0,1}
    w1_p = np.zeros((P, (KF1 // 2) * KH * 2 * P), np.float32)
    for t in range(KF1 // 2):
        for m in range(KH):
            for h in range(2):
                kk = 2 * t + h
                w1_p[:, ((t * KH + m) * 2 + h) * P:((t * KH + m) * 2 + h + 1) * P] = \
                    w1[kk * P:(kk + 1) * P, m * P:(m + 1) * P]
    w1_p = _f8(w1_p, SW)
    w2_p = np.zeros((P, (KH // 2) * KH * 2 * P), np.float32)
    for t in range(KH // 2):
        for m in range(KH):
            for h in range(2):
                kk = 2 * t + h
                w2_p[:, ((t * KH + m) * 2 + h) * P:((t * KH + m) * 2 + h + 1) * P] = \
                    w2[kk * P:(kk + 1) * P, m * P:(m + 1) * P]
    w2_p = _f8(w2_p, SW)
    # w3 rhs: [P, (t, h, 4)]
    w3pad = np.pad(w3, ((0, 0), (0, 1)))
    w3_p = np.zeros((P, (KH // 2) * 2 * 4), np.float32)
    for t in range(KH // 2):
        for h in range(2):
            kk = 2 * t + h
            w3_p[:, (t * 2 + h) * 4:(t * 2 + h + 1) * 4] = \
                w3pad[kk * P:(kk + 1) * P, :]
    w3_p = _f8(w3_p, SW)
    b3_p = np.pad(b3, (0, 1)).reshape(1, 4).repeat(P, 0) * (SW * SH)

    # g1z fused rhs: [P, (t, h, [g1w*SW | W13*SZ])]
    g1z_p = np.zeros((P, (KF1 // 2) * 2 * 2 * G1), np.float32)
    for t in range(KF1 // 2):
        for h in range(2):
            kk = 2 * t + h
            base = (t * 2 + h) * 2 * G1
            g1z_p[:, base:base + G1] = g1w[kk * P:(kk + 1) * P, :] * SW
            g1z_p[:, base + G1:base + 2 * G1] = W13[kk * P:(kk + 1) * P, :] * SZ
    g1z_p = g1z_p.astype(F8)
    # g2w rhs: [P, (h, G2)] bf16*SW (r2 path stays bf16)
    g2w_p = np.zeros((P, 2 * G2), np.float32)
    for h in range(2):
        g2w_p[:, h * G2:(h + 1) * G2] = g2w[h * P:(h + 1) * P, :] * SW
    g2w_p = _bf(g2w_p)

    def pack_k3(w, F, s=1.0):
        o = np.zeros((4, F), np.float32)
        o[:3] = w * s
        return _bf(o)

    W1s_p = pack_k3(W1s, cfg.F1, SR1)
    W12_p = pack_k3(W12, G1)
    W2s_p = pack_k3(W2s, G1)
    fcw_p = _bf(fcw)

    def pack_dinv(dv, s):
        t = np.zeros((P, NB), np.float32)
        t.T.reshape(-1)[:cfg.NLOC_RAW] = dv * s
        return t

    in_maps = []
    for c in range(NC):
        lo = c * cfg.NLOC_RAW
        hi = lo + cfg.NLOC_RAW
        dv = dinv[lo:hi]
        dinv_a = pack_dinv(dv, 1.0 / ST1)         # g1 finalize
        dinv_c = pack_dinv(dv, SR2)               # r2 creation
        dinv_d = pack_dinv(dv, ST1 / (SR1 * SW))  # h1 table creation
        dinv_e = pack_dinv(dv, 1.0 / (SR2 * SW))  # out finalize
        x1T = np.zeros((cfg.F1, cfg.NLOC), np.float32)
        x1T[:, :cfg.NLOC_RAW] = x1[lo:hi].T
        x2T = np.zeros((cfg.F1, cfg.NLOC), np.float32)
        x2T[:, :cfg.NLOC_RAW] = x2[lo:hi].T

        i1 = [pack_round(lay1[r], pc1[r][c], cfg.SBP1, NB) for r in range(3)]
        i2 = [pack_round(lay2[r], pc2[r][c], cfg.SBP2, NB) for r in range(2)]
        idx1 = np.concatenate([a for a, _ in i1], axis=0)
        Sm1 = np.concatenate([b for _, b in i1], axis=0)
        idx2 = np.concatenate([a for a, _ in i2], axis=0)
        Sm2 = np.concatenate([b for _, b in i2], axis=0)

        im = {
            "ident": _bf(np.eye(P)),
            "x1T": _f8(x1T), "x2T": _f8(x2T),
            "idx1": idx1, "Sm1": Sm1, "idx2": idx2, "Sm2": Sm2,
            "dinv_a": dinv_a, "dinv_c": dinv_c,
            "dinv_d": dinv_d, "dinv_e": dinv_e,
            "w1": w1_p, "w2": w2_p, "w3": w3_p, "b3": b3_p.astype(np.float32),
            "g1z": g1z_p, "g2w": g2w_p,
            "W1s": W1s_p, "W12": W12_p, "W2s": W2s_p, "fcw": fcw_p,
        }
        in_maps.append(im)
    return in_maps, sched


def build(cfg, sched, debug=False, dbg_taps=False):
    import concourse.bacc as bacc
    import concourse.bass as bass
    import concourse.mybir as mybir
    import concourse.tile as tile

    dt = mybir.dt
    AF = mybir.ActivationFunctionType
    OP = mybir.AluOpType
    AX = mybir.AxisListType
    DR = mybir.MatmulPerfMode.DoubleRow

    nc = bacc.Bacc("TRN2", target_bir_lowering=False, debug=debug)

    NB, C, BPC, CH, NLOC, CHR = (cfg.NB, cfg.C, cfg.BPC, cfg.CH,
                                 cfg.NLOC, cfg.CHR)
    F1, H, G1, G2, FOUT = cfg.F1, cfg.H, cfg.G1, cfg.G2, cfg.FOUT
    KF1, KH = cfg.KF1, cfg.KH
    NF, FU = cfg.NFREE, cfg.FU
    SBP1, SBP2 = cfg.SBP1, cfg.SBP2
    SPL2 = cfg.SPL2
    SPL2B = SPL2 * BPC * P
    NL2B = NLOC - SPL2B
    L1A, L1B, L1C = sched["lay1"]
    L2A, L2B = sched["lay2"]
    SB1 = L1A["nsb"] + L1B["nsb"] + L1C["nsb"]
    SB2 = L2A["nsb"] + L2B["nsb"]

    bf = dt.bfloat16
    f8 = dt.float8e4
    f32 = dt.float32

    def din(name, shape, dtype):
        return nc.declare_dram_parameter(name, list(shape), dtype,
                                         isOutput=False)

    x1T_d = din("x1T", [F1, NLOC], f8)
    x2T_d = din("x2T", [F1, NLOC], f8)
    idx1_d = din("idx1", [SB1 * P, 2 * SBP1], dt.int32)
    Sm1_d = din("Sm1", [SB1 * P, 2 * SBP1 * P], f8)
    idx2_d = din("idx2", [SB2 * P, 2 * SBP2], dt.int32)
    Sm2_d = din("Sm2", [SB2 * P, 2 * SBP2 * P], f8)
    dinva_d = din("dinv_a", [P, NB], f32)
    dinvc_d = din("dinv_c", [P, NB], f32)
    dinvd_d = din("dinv_d", [P, NB], f32)
    dinve_d = din("dinv_e", [P, NB], f32)
    w1_d = din("w1", [P, (KF1 // 2) * KH * 2 * P], f8)
    w2_d = din("w2", [P, (KH // 2) * KH * 2 * P], f8)
    w3_d = din("w3", [P, (KH // 2) * 2 * 4], f8)
    b3_d = din("b3", [P, 4], f32)
    g1z_d = din("g1z", [P, (KF1 // 2) * 2 * 2 * G1], f8)
    g2w_d = din("g2w", [P, 2 * G2], bf)
    W1s_d = din("W1s", [4, F1], bf)
    W12_d = din("W12", [4, G1], bf)
    W2s_d = din("W2s", [4, G1], bf)
    fcw_d = din("fcw", [G2, FOUT], bf)
    ident_d = din("ident", [P, P], bf)
    out_d = nc.declare_dram_parameter("out", [NLOC, FOUT], f32, isOutput=True)
    if dbg_taps:
        dbg_h2bA = nc.declare_dram_parameter("dbg_h2bA", [SPL2B, G2], f8, isOutput=True)
        dbg_h2gA = nc.declare_dram_parameter("dbg_h2gA", [cfg.NC * SPL2B, G2], f8, isOutput=True)
        dbg_h2gB = nc.declare_dram_parameter("dbg_h2gB", [cfg.NC * NL2B, G2], f8, isOutput=True)
        dbg_mT = nc.declare_dram_parameter("dbg_mT", [4, NLOC], bf, isOutput=True)
        dbg_z = nc.declare_dram_parameter("dbg_z", [P, NB * G1], bf, isOutput=True)
        dbg_aggA = nc.declare_dram_parameter("dbg_aggA", [P, NB * G1], bf, isOutput=True)
        dbg_agg2T = nc.declare_dram_parameter("dbg_agg2T", [G2, NB * P], bf, isOutput=True)
        dbg_oacc = nc.declare_dram_parameter("dbg_oacc", [P, NB * FOUT], f32, isOutput=True)

    with tile.TileContext(nc) as tc:
        with (
            tc.tile_pool(name="const", bufs=1) as cp,
            tc.tile_pool(name="front", bufs=2) as fp,
            tc.tile_pool(name="scat", bufs=2) as sp,
            tc.tile_pool(name="fin", bufs=3) as qp,
            tc.tile_pool(name="psG", bufs=2, space="PSUM") as psG,
            tc.tile_pool(name="psS", bufs=2, space="PSUM") as psS,
            tc.tile_pool(name="psW", bufs=2, space="PSUM") as psW,
            tc.tile_pool(name="psT", bufs=1, space="PSUM") as psT,
            tc.tile_pool(name="ps2", bufs=1, space="PSUM") as ps2p,
            tc.tile_pool(name="dram", bufs=1, space="DRAM") as dp,
        ):
            def load(dr, shape, dtype, name):
                t = cp.tile(shape, dtype, tag=name, name=name + "_s")
                nc.sync.dma_start(out=t[:, :], in_=dr[:, :])
                return t

            w1_s = load(w1_d, [P, (KF1 // 2) * KH * 2 * P], f8, "w1")
            w2_s = load(w2_d, [P, (KH // 2) * KH * 2 * P], f8, "w2")
            w3_s = load(w3_d, [P, (KH // 2) * 2 * 4], f8, "w3")
            b3_s = load(b3_d, [P, 4], f32, "b3")
            g1z_s = load(g1z_d, [P, (KF1 // 2) * 2 * 2 * G1], f8, "g1z")
            g2w_s = load(g2w_d, [P, 2 * G2], bf, "g2w")
            W1s_s = load(W1s_d, [4, F1], bf, "W1s")
            W12_s = load(W12_d, [4, G1], bf, "W12")
            W2s_s = load(W2s_d, [4, G1], bf, "W2s")
            fcw_s = load(fcw_d, [G2, FOUT], bf, "fcw")
            dinva_s = load(dinva_d, [P, NB], f32, "dinva")
            dinvc_s = load(dinvc_d, [P, NB], f32, "dinvc")
            dinvd_s = load(dinvd_d, [P, NB], f32, "dinvd")
            dinve_s = load(dinve_d, [P, NB], f32, "dinve")
            ident = load(ident_d, [P, P], bf, "ident")

            mT_s = cp.tile([4, NLOC], bf, tag="mT")
            z_s = cp.tile([P, NB * G1], f8, tag="z")
            aggA_s = cp.tile([P, NB * G1], bf, tag="aggA")
            agg2T_s = cp.tile([G2, NB * P], bf, tag="agg2T")
            out_acc = cp.tile([P, NB * FOUT], f32, tag="oacc")

            h1bs = [dp.tile([(k1 - k0) * CH, G1], f8, tag=f"h1b{r}",
                            name=f"h1b{r}")
                    for r, (k0, k1) in enumerate(cfg.L1R)]
            h1g = [
                dp.tile([(k1 - k0) * CHR + P, G1], f8, tag=f"h1g{r}",
                        name=f"h1g{r}")
                for r, (k0, k1) in enumerate(cfg.L1R)]
            h2bA = dp.tile([SPL2B, G2], f8, tag="h2bA")
            h2bB = dp.tile([NL2B, G2], f8, tag="h2bB")
            h2g = [
                dp.tile([cfg.NC * SPL2B + P, G2], f8, tag="h2gA", name="h2gA"),
                dp.tile([cfg.NC * NL2B + P, G2], f8, tag="h2gB", name="h2gB"),
            ]


            # The Collectives semaphore increments when the CC engine
            # STARTS a collective, and the CC queue is serial — so a
            # tiny sentinel AllGather into the table's spare tail rows
            # gives consumers (which depend on the whole table tensor) a
            # wait that only clears after the REAL collective's data has
            # fully landed.
            def sentinel(src_buf, table, nrows):
                return nc.gpsimd.collective_compute(
                    "AllGather", OP.bypass,
                    replica_groups=[list(range(cfg.NC))],
                    ins=[src_buf[0:P // cfg.NC, :].opt()],
                    outs=[table[nrows:nrows + P, :].opt()])

            # ================= FRONT (per chunk) =================
            def front_chunk(k):
                n0 = k * CH
                x1c = fp.tile([P, KF1 * CH], f8, tag="x1c")
                nc.sync.dma_start(
                    out=x1c[:, :].rearrange("p (a n) -> p a n", n=CH),
                    in_=x1T_d[:, n0:n0 + CH].rearrange("(a p) n -> p a n", p=P))
                x2c = fp.tile([P, KF1 * CH], f8, tag="x2c")
                nc.sync.dma_start(
                    out=x2c[:, :].rearrange("p (a n) -> p a n", n=CH),
                    in_=x2T_d[:, n0:n0 + CH].rearrange("(a p) n -> p a n", p=P))

                h1T = fp.tile([P, KH * CH], f8, tag="h1T")
                for u in range(FU):
                    for m in range(KH):
                        ps = psG.tile([P, 512], f32, tag="g")
                        for t in range(KF1 // 2):
                            nc.tensor.matmul(
                                ps[:, :NF],
                                lhsT=w1_s[:, (t * KH + m) * 2 * P:
                                          (t * KH + m + 1) * 2 * P]
                                .rearrange("p (a q) -> p a q", a=2),
                                rhs=x1c[:, :]
                                .rearrange("p (a n) -> p a n", n=CH)
                                [:, 2 * t:2 * t + 2, u * NF:u * NF + NF],
                                start=(t == 0), stop=(t == KF1 // 2 - 1),
                                perf_mode=DR)
                        nc.scalar.activation(
                            h1T[:, m * CH + u * NF:m * CH + u * NF + NF],
                            ps[:, :NF], AF.Relu, scale=1.0 / SW)
                h2T = fp.tile([P, KH * CH], f8, tag="h2T")
                for u in range(FU):
                    for m in range(KH):
                        ps = psG.tile([P, 512], f32, tag="g")
                        for t in range(KH // 2):
                            nc.tensor.matmul(
                                ps[:, :NF],
                                lhsT=w2_s[:, (t * KH + m) * 2 * P:
                                          (t * KH + m + 1) * 2 * P]
                                .rearrange("p (a q) -> p a q", a=2),
                                rhs=h1T[:, :]
                                .rearrange("p (a n) -> p a n", n=CH)
                                [:, 2 * t:2 * t + 2, u * NF:u * NF + NF],
                                start=(t == 0), stop=(t == KH // 2 - 1),
                                perf_mode=DR)
                        nc.scalar.activation(
                            h2T[:, m * CH + u * NF:m * CH + u * NF + NF],
                            ps[:, :NF], AF.Relu, scale=1.0 / SW)

                mmc = fp.tile([P, BPC * 3], bf, tag="mmc")
                for nb in range(BPC):
                    psl = psW.tile([P, 512], f32, tag="b")
                    for t in range(KH // 2):
                        nc.tensor.matmul(
                            psl[:, :4],
                            lhsT=h2T[:, :]
                            .rearrange("p (a n) -> p a n", n=CH)
                            [:, 2 * t:2 * t + 2, nb * P:(nb + 1) * P],
                            rhs=w3_s[:, t * 8:(t + 1) * 8]
                            .rearrange("p (a q) -> p a q", a=2),
                            start=(t == 0), stop=(t == KH // 2 - 1),
                            perf_mode=DR)
                    lg = fp.tile([P, 3], f32, tag="lg")
                    if sched["b3_nz"]:
                        nc.vector.tensor_add(lg[:, :], psl[:, :3], b3_s[:, :3])
                    else:
                        nc.vector.tensor_copy(lg[:, :], psl[:, :3])
                    rmax = fp.tile([P, 1], f32, tag="rmax")
                    nc.vector.reduce_max(rmax[:, :], lg[:, :], axis=AX.X)
                    mm = fp.tile([P, 3], bf, tag="mm")
                    nc.vector.tensor_scalar(
                        mm[:, :], lg[:, :], rmax[:, :1], None, OP.is_equal)
                    nc.scalar.activation(mmc[:, nb * 3:(nb + 1) * 3],
                                         mm[:, :], AF.Copy,
                                         bias=TAU_LO, scale=TAU_HI - TAU_LO)
                for nb in range(BPC):
                    b_glob = k * BPC + nb
                    pst = psT.tile([P, P], bf, tag="t")
                    nc.tensor.transpose(pst[:3, :],
                                        mmc[:, nb * 3:(nb + 1) * 3],
                                        ident[:, :])
                    nc.vector.tensor_copy(
                        mT_s[:3, b_glob * P:(b_glob + 1) * P], pst[:3, :])

                # r1 = (m@W1s)*x2, fp8*SR1 (SR1 folded into W1s)
                r1T = fp.tile([P, KF1 * CH], f8, tag="r1T")
                for u in range(FU):
                    for f in range(KF1):
                        psr = psG.tile([P, 512], f32, tag="g")
                        nc.tensor.matmul(
                            psr[:, :NF], lhsT=W1s_s[:3, f * P:(f + 1) * P],
                            rhs=mT_s[:3, n0 + u * NF:n0 + u * NF + NF],
                            start=True, stop=True)
                        nc.vector.tensor_mul(
                            r1T[:, f * CH + u * NF:f * CH + u * NF + NF],
                            psr[:, :NF],
                            x2c[:, f * CH + u * NF:f * CH + u * NF + NF])

                # g1 conv + w13 fused rhs -> h1 table + z
                for nb in range(BPC):
                    b_glob = k * BPC + nb
                    psh = psG.tile([P, 512], f32, tag="g")
                    for t in range(KF1 // 2):
                        nc.tensor.matmul(
                            psh[:, :],
                            lhsT=r1T[:, :]
                            .rearrange("p (a n) -> p a n", n=CH)
                            [:, 2 * t:2 * t + 2, nb * P:(nb + 1) * P],
                            rhs=g1z_s[:, t * 2 * 2 * G1:(t + 1) * 2 * 2 * G1]
                            .rearrange("p (a q) -> p a q", a=2),
                            start=(t == 0), stop=(t == KF1 // 2 - 1),
                            perf_mode=DR)
                    h1p = fp.tile([P, G1], f8, tag="h1p")
                    nc.scalar.activation(h1p[:, :], psh[:, :G1], AF.Copy,
                                         scale=dinvd_s[:, b_glob:b_glob + 1])
                    r = next(i for i, (a0, a1) in enumerate(cfg.L1R)
                             if a0 <= k < a1)
                    koff = k - cfg.L1R[r][0]
                    row0 = (koff * BPC + nb) * P
                    nc.scalar.dma_start(
                        out=h1bs[r][row0:row0 + P, :], in_=h1p[:, :])
                    nc.scalar.activation(
                        z_s[:, b_glob * G1:(b_glob + 1) * G1],
                        psh[:, G1:2 * G1], AF.Copy,
                        scale=4096.0 / (SZ * SR1))

                # one AllGather per round table, after its last chunk
                r = next(i for i, (a0, a1) in enumerate(cfg.L1R)
                         if a0 <= k < a1)
                if k == cfg.L1R[r][1] - 1:
                    nrows = (cfg.L1R[r][1] - cfg.L1R[r][0]) * CHR
                    nc.gpsimd.collective_compute(
                        "AllGather", OP.bypass,
                        replica_groups=[list(range(cfg.NC))],
                        ins=[h1bs[r][:, :].opt()],
                        outs=[h1g[r][0:nrows, :].opt()])
                    sentinel(h1bs[r], h1g[r], nrows)

            # ================= L1 scatter =================
            ps_by_b = {}

            def l1_finalize(b):
                psb = ps_by_b.pop(b)
                sum_bf = qp.tile([P, G1], bf, tag="sumb")
                nc.vector.tensor_add(sum_bf[:, :], psb[:, :],
                                     aggA_s[:, b * G1:(b + 1) * G1])
                g1r = qp.tile([P, G1], bf, tag="g1r")
                nc.scalar.activation(g1r[:, :], sum_bf[:, :], AF.Relu,
                                     scale=dinva_s[:, b:b + 1])
                if sched["w12_ones"]:
                    g1v = qp.tile([P, G1], bf, tag="g1v")
                    nc.vector.scalar_tensor_tensor(
                        out=g1v[:, :], in0=z_s[:, b * G1:(b + 1) * G1],
                        scalar=1.0 / 4096.0, in1=g1r[:, :],
                        op0=OP.mult, op1=OP.add)
                else:
                    psw12 = psW.tile([P, 512], f32, tag="b")
                    nc.tensor.matmul(psw12[:, :G1],
                                     lhsT=mT_s[:3, b * P:(b + 1) * P],
                                     rhs=W12_s[:3, :], start=True, stop=True)
                    g1t = qp.tile([P, G1], bf, tag="g1t")
                    nc.vector.tensor_mul(g1t[:, :], g1r[:, :], psw12[:, :G1])
                    g1v = qp.tile([P, G1], bf, tag="g1v")
                    nc.vector.scalar_tensor_tensor(
                        out=g1v[:, :], in0=z_s[:, b * G1:(b + 1) * G1],
                        scalar=1.0 / 4096.0, in1=g1t[:, :],
                        op0=OP.mult, op1=OP.add)
                psmw = psW.tile([P, 512], f32, tag="b")
                nc.tensor.matmul(psmw[:, :G1],
                                 lhsT=mT_s[:3, b * P:(b + 1) * P],
                                 rhs=W2s_s[:3, :], start=True, stop=True)
                r2q = qp.tile([P, G1], bf, tag="r2q")
                nc.vector.scalar_tensor_tensor(
                    out=r2q[:, :], in0=psmw[:, :G1],
                    scalar=dinvc_s[:, b:b + 1],
                    in1=g1v[:, :], op0=OP.mult, op1=OP.mult)
                r2T = qp.tile([P, 2 * P], bf, tag="r2T")
                for f in range(2):
                    pst = psT.tile([P, P], bf, tag="t", name="pst8")
                    nc.tensor.transpose(pst[:, :], r2q[:, f * P:(f + 1) * P],
                                        ident[:, :])
                    nc.scalar.activation(r2T[:, f * P:(f + 1) * P],
                                         pst[:, :], AF.Copy)
                psh2 = psW.tile([P, 512], f32, tag="b")
                for f in range(2):
                    nc.tensor.matmul(
                        psh2[:, :G2],
                        lhsT=r2T[:, f * P:(f + 1) * P],
                        rhs=g2w_s[:, f * G2:(f + 1) * G2],
                        start=(f == 0), stop=(f == 1))
                h2p = qp.tile([P, G2], f8, tag="h2p")
                nc.scalar.activation(h2p[:, :], psh2[:, :G2], AF.Copy)
                if b < SPL2 * BPC:
                    nc.scalar.dma_start(
                        out=h2bA[b * P:(b + 1) * P, :], in_=h2p[:, :])
                    if b == SPL2 * BPC - 1:
                        nc.gpsimd.collective_compute(
                            "AllGather", OP.bypass,
                            replica_groups=[list(range(cfg.NC))],
                            ins=[h2bA[:, :].opt()],
                            outs=[h2g[0][0:cfg.NC * SPL2B, :].opt()])
                        sentinel(h2bA, h2g[0], cfg.NC * SPL2B)
                else:
                    bb = b - SPL2 * BPC
                    nc.scalar.dma_start(
                        out=h2bB[bb * P:(bb + 1) * P, :], in_=h2p[:, :])
                    if b == NB - 1:
                        nc.gpsimd.collective_compute(
                            "AllGather", OP.bypass,
                            replica_groups=[list(range(cfg.NC))],
                            ins=[h2bB[:, :].opt()],
                            outs=[h2g[1][0:cfg.NC * NL2B, :].opt()])
                        sentinel(h2bB, h2g[1], cfg.NC * NL2B)

            def l1_sblock(meta, r, sb_base, s_loc, table):
                """Issue one L1 s-block (SBP1 pairs)."""
                s = sb_base + s_loc
                ix = sp.tile([P, 2 * SBP1], dt.int32, tag="ix1")
                nc.sync.dma_start(out=ix[:, :],
                                  in_=idx1_d[s * P:(s + 1) * P, :])
                gt = sp.tile([P, 2 * SBP1 * G1], f8, tag="gt1", bufs=3)
                nc.gpsimd.indirect_dma_start(
                    out=gt[:, :], out_offset=None, in_=table[:, :],
                    in_offset=bass.IndirectOffsetOnAxis(ap=ix[:, :], axis=0))
                Ssb = sp.tile([P, 2 * SBP1 * P], f8, tag="S1", bufs=3)
                nc.sync.dma_start(out=Ssb[:, :],
                                  in_=Sm1_d[s * P:(s + 1) * P, :])
                for jp in range(SBP1):
                    g = s_loc * SBP1 + jp
                    b = int(meta["b_of"][g])
                    first = bool(meta["first"][g])
                    last = bool(meta["last"][g])
                    if first:
                        ps_by_b[b] = psS.tile([P, G1], f32, tag="agg",
                                              name="agg1")
                    psb = ps_by_b[b]
                    nc.tensor.matmul(
                        psb[:, :],
                        lhsT=Ssb[:, jp * 2 * P:(jp + 1) * 2 * P]
                        .rearrange("p (a q) -> p a q", a=2),
                        rhs=gt[:, jp * 2 * G1:(jp + 1) * 2 * G1]
                        .rearrange("p (a q) -> p a q", a=2),
                        start=first, stop=last, perf_mode=DR)
                    if not last:
                        continue
                    if r == 0:
                        nc.vector.tensor_copy(
                            aggA_s[:, b * G1:(b + 1) * G1],
                            ps_by_b.pop(b)[:, :])
                    elif r == 1:
                        nc.vector.tensor_add(
                            aggA_s[:, b * G1:(b + 1) * G1],
                            ps_by_b.pop(b)[:, :],
                            aggA_s[:, b * G1:(b + 1) * G1])
                    else:
                        l1_finalize(b)

            # ================= L2 scatter =================
            ps2 = {}

            def l2_finalize(b):
                psb2 = ps2.pop(b)
                sum2 = qp.tile([G2, P], bf, tag="sum2")
                nc.vector.tensor_add(sum2[:, :], psb2[:, :],
                                     agg2T_s[:, b * P:(b + 1) * P])
                g2T = qp.tile([G2, P], bf, tag="g2T")
                nc.scalar.activation(g2T[:, :], sum2[:, :], AF.Relu)
                psf = psW.tile([P, 512], f32, tag="b")
                nc.tensor.matmul(psf[:, :FOUT], lhsT=g2T[:, :],
                                 rhs=fcw_s[:, :], start=True, stop=True)
                nc.scalar.activation(
                    out_acc[:, b * FOUT:(b + 1) * FOUT], psf[:, :FOUT],
                    AF.Copy, scale=dinve_s[:, b:b + 1])

            def l2_sblock(meta, u, sb_base, s_loc, table):
                s = sb_base + s_loc
                ix2 = sp.tile([P, 2 * SBP2], dt.int32, tag="ix2")
                nc.sync.dma_start(out=ix2[:, :],
                                  in_=idx2_d[s * P:(s + 1) * P, :])
                gt2 = sp.tile([P, 2 * SBP2 * G2], f8, tag="gt2", bufs=3)
                nc.gpsimd.indirect_dma_start(
                    out=gt2[:, :], out_offset=None, in_=table[:, :],
                    in_offset=bass.IndirectOffsetOnAxis(ap=ix2[:, :], axis=0))
                S2 = sp.tile([P, 2 * SBP2 * P], f8, tag="S2", bufs=3)
                nc.sync.dma_start(out=S2[:, :],
                                  in_=Sm2_d[s * P:(s + 1) * P, :])
                for jp in range(SBP2):
                    g = s_loc * SBP2 + jp
                    b = int(meta["b_of"][g])
                    first = bool(meta["first"][g])
                    last = bool(meta["last"][g])
                    if first:
                        ps2[b] = ps2p.tile([G2, P], f32, tag="agg2",
                                           name="agg2")
                    psb2 = ps2[b]
                    nc.tensor.matmul(
                        psb2[:, :],
                        lhsT=gt2[:, jp * 2 * G2:(jp + 1) * 2 * G2]
                        .rearrange("p (a q) -> p a q", a=2),
                        rhs=S2[:, jp * 2 * P:(jp + 1) * 2 * P]
                        .rearrange("p (a q) -> p a q", a=2),
                        start=first, stop=last, perf_mode=DR)
                    if not last:
                        continue
                    if u == 0:
                        nc.vector.tensor_copy(
                            agg2T_s[:, b * P:(b + 1) * P],
                            ps2.pop(b)[:, :])
                    else:
                        l2_finalize(b)

            # ======== program order with interleaving ========
            nsb0 = L1A["nsb"]
            for k in range(4):
                front_chunk(k)
            quota = [0, nsb0 // 2, nsb0 - nsb0 // 2]
            done0 = 0
            for k in range(4, 7):
                front_chunk(k)
                for s_loc in range(done0, done0 + quota[k - 4]):
                    l1_sblock(L1A, 0, 0, s_loc, h1g[0])
                done0 += quota[k - 4]
            done2 = 0
            for s_loc in range(L1B["nsb"]):
                l1_sblock(L1B, 1, nsb0, s_loc, h1g[1])
                while (done2 < L1C["nsb"] and s_loc >= 2 * done2 + 3):
                    l1_sblock(L1C, 2, nsb0 + L1B["nsb"], done2, h1g[2])
                    done2 += 1
            if dbg_taps:
                nc.scalar.dma_start(out=dbg_mT[:, :], in_=mT_s[:, :])
                nc.scalar.dma_start(out=dbg_z[:, :], in_=z_s[:, :])
                nc.scalar.dma_start(out=dbg_aggA[:, :], in_=aggA_s[:, :])
            for s_loc in range(done2, L1C["nsb"]):
                l1_sblock(L1C, 2, nsb0 + L1B["nsb"], s_loc, h1g[2])

            for s_loc in range(L2A["nsb"]):
                l2_sblock(L2A, 0, 0, s_loc, h2g[0])
            if dbg_taps:
                nc.scalar.dma_start(out=dbg_agg2T[:, :], in_=agg2T_s[:, :])
                for part in range(8):
                    rows = cfg.NC * SPL2B // 8
                    stg2 = qp.tile([P, rows * G2 // P], f8, tag="stg2",
                                   bufs=2, name=f"stg2_{part}")
                    nc.sync.dma_start(
                        out=stg2[:, :],
                        in_=h2g[0][part * rows:(part + 1) * rows, :]
                        .rearrange("(p a) g -> p (a g)", p=P))
                    nc.sync.dma_start(
                        out=dbg_h2gA[part * rows:(part + 1) * rows, :]
                        .rearrange("(p a) g -> p (a g)", p=P),
                        in_=stg2[:, :])
                for part in range(8):
                    rows = cfg.NC * NL2B // 8
                    stg3 = qp.tile([P, rows * G2 // P], f8, tag="stg2",
                                   bufs=2, name=f"stg3_{part}")
                    nc.sync.dma_start(
                        out=stg3[:, :],
                        in_=h2g[1][part * rows:(part + 1) * rows, :]
                        .rearrange("(p a) g -> p (a g)", p=P))
                    nc.sync.dma_start(
                        out=dbg_h2gB[part * rows:(part + 1) * rows, :]
                        .rearrange("(p a) g -> p (a g)", p=P),
                        in_=stg3[:, :])
                for part in range(5):
                    rows = SPL2B // 5
                    stg4 = qp.tile([P, rows * G2 // P], f8, tag="stg2",
                                   bufs=2, name=f"stg4_{part}")
                    nc.sync.dma_start(
                        out=stg4[:, :],
                        in_=h2bA[part * rows:(part + 1) * rows, :]
                        .rearrange("(p a) g -> p (a g)", p=P))
                    nc.sync.dma_start(
                        out=dbg_h2bA[part * rows:(part + 1) * rows, :]
                        .rearrange("(p a) g -> p (a g)", p=P),
                        in_=stg4[:, :])

            for s_loc in range(L2B["nsb"]):
                l2_sblock(L2B, 1, L2A["nsb"], s_loc, h2g[1])

            if dbg_taps:
                nc.scalar.dma_start(out=dbg_oacc[:, :], in_=out_acc[:, :])
            # batched log_softmax (logits tiny: exp without max-shift safe)
            e_all = qp.tile([P, NB * FOUT], f32, tag="eall", bufs=1)
            nc.scalar.activation(e_all[:, :], out_acc[:, :], AF.Exp)
            sums = qp.tile([P, NB], f32, tag="sums", bufs=1)
            nc.vector.reduce_sum(
                sums[:, :],
                e_all[:, :].rearrange("p (b f) -> p b f", f=FOUT),
                axis=AX.X)
            lns = qp.tile([P, NB], f32, tag="lns", bufs=1)
            nc.scalar.activation(lns[:, :], sums[:, :], AF.Ln)
            res = qp.tile([P, NB * FOUT], f32, tag="eall", bufs=1, name="res")
            nc.vector.tensor_tensor(
                out=res[:, :].rearrange("p (b f) -> p b f", f=FOUT),
                in0=out_acc[:, :].rearrange("p (b f) -> p b f", f=FOUT),
                in1=lns[:, :].unsqueeze(2).to_broadcast([P, NB, FOUT]),
                op=OP.subtract)
            nc.scalar.dma_start(
                out=out_d[:, :].rearrange("(b p) f -> p b f", p=P),
                in_=res[:, :].rearrange("p (b f) -> p b f", f=FOUT))
    return nc


_LAST_EXEC_NS = None
_LAST_RESULT = None


def run(inputs, cfg, trace=False, debug=False, dbg_taps=False):
    global _LAST_EXEC_NS, _LAST_RESULT
    in_maps, sched = host_prep(inputs, cfg)
    nc = build(cfg, sched, debug=debug, dbg_taps=dbg_taps)
    nc.finalize()
    from concourse import bass_utils
    res = bass_utils.run_bass_kernel_spmd(
        nc, in_maps, core_ids=list(range(cfg.NC)), trace=trace)
    _LAST_EXEC_NS = res.exec_time_ns
    _LAST_RESULT = res
    outs = [np.asarray(res.results[c]["out"])[:cfg.NLOC_RAW]
            for c in range(cfg.NC)]
    return np.concatenate(outs, 0).astype(np.float32)


def kernel(**inputs):
    return run(inputs, _Cfg(**CFG_FULL))
